# revision 36
# baseline (speedup 1.0000x reference)
"""Self-contained Trainium2 Bass kernel for NonLocalMeansFast.

Full inputs in, full output out; internally shards across 8 NeuronCores.

Strategy (stripe layout, all compute lane-local):
  - core c owns output rows [64c, 64c+64)
  - partitions p in 0..127: row r = 64c + p%64, x-half = p//64
  - 2 chunk-steps over x: xbase(p,s) = 256*(p//64) + 128*s, 128 cols each
  - per-partition DMA-replicated neighborhoods (y: 15 rows x 152, rgb: 11 x 138)
  - all 121 shifts: weights w_a = exp(-sqrt(box5((y - shift_a(y))^2))/h)
    computed as windowed wide-instruction groups (11 x-shifts per instruction)
  - VectorE: diffs/box-trees/products/reduces; ScalarE: square/sqrt/exp
"""

import sys
import os
import numpy as np

for _p in ("/opt/trn_rl_repo", "/root/.axon_site/_ro/trn_rl_repo"):
    if os.path.isdir(_p) and _p not in sys.path:
        sys.path.insert(0, _p)

import concourse.bass as bass
import concourse.bacc as bacc
import concourse.tile as tile
from concourse import mybir
from concourse.bass_utils import run_bass_kernel_spmd

SR, PR = 5, 2
H = W = 512
NCORES = 8
CHUNK = 128
NSTEP = 2
YN_W = CHUNK + 2 * (SR + PR + SR)     # 152
YN_K = 2 * (SR + PR) + 1              # 15
RN_W = CHUNK + 2 * SR                 # 138
RN_K = 2 * SR + 1                     # 11
D_W = CHUNK + 2 * SR                  # 138
D2_W = CHUNK + 2 * (SR + PR)          # 142
RGBX_W = W + 24                       # 536
RGBX_K = 64 + 14                      # 78

# 11 groups of 11 x-shift windows; ar=0 includes the (0,0) center shift,
# whose map is exactly 0 -> w = exp(0) = 1, matching the reference's center.
GROUPS = [(ar, -SR, 2 * SR + 1) for ar in range(-SR, SR + 1)]
NMAPS = sum(g[2] for g in GROUPS)     # 121

F32 = mybir.dt.float32
DT = mybir.dt.float16                 # compute dtype for maps/products
ALU = mybir.AluOpType
ACTF = mybir.ActivationFunctionType


def _vp(ap, dims, offset):
    """Return a copy of `ap` with a custom [step,count] pattern + offset."""
    c = ap.copy()
    c.ap = mybir.VecI64Pair([(int(s), int(n)) for s, n in dims])
    c.offset = int(offset)
    return c


def build_bass(repeat=1):
    nc = bacc.Bacc("TRN2", target_bir_lowering=False, debug=False)

    rgbx = nc.dram_tensor("rgbx", [3, RGBX_K, RGBX_W], DT, kind="ExternalInput")
    sig = nc.dram_tensor("sigma", [1, 1], F32, kind="ExternalInput")
    outd = nc.dram_tensor("out", [3, 64, W], F32, kind="ExternalOutput")

    RGBX_P = RGBX_K * RGBX_W  # flat pitch of one channel in rgbx

    with tile.TileContext(nc) as tc:
        import contextlib
        with contextlib.ExitStack() as ctx:
            const_p = ctx.enter_context(tc.tile_pool(name="const", bufs=1))
            rgb78_p = ctx.enter_context(tc.tile_pool(name="rgb78", bufs=2))
            y78_p = ctx.enter_context(tc.tile_pool(name="y78", bufs=1))
            yn_p = ctx.enter_context(tc.tile_pool(name="yn", bufs=2))
            rgbn_p = ctx.enter_context(tc.tile_pool(name="rgbn", bufs=2))
            diff_p = ctx.enter_context(tc.tile_pool(name="diff", bufs=4))
            d2_p = ctx.enter_context(tc.tile_pool(name="d2", bufs=4))
            box_p = ctx.enter_context(tc.tile_pool(name="box", bufs=3))
            dbuf_p = ctx.enter_context(tc.tile_pool(name="dbuf", bufs=2))
            t_p = ctx.enter_context(tc.tile_pool(name="tt", bufs=1))
            red_p = ctx.enter_context(tc.tile_pool(name="red", bufs=2))
            acc_p = ctx.enter_context(tc.tile_pool(name="acc", bufs=2))
            fin_p = ctx.enter_context(tc.tile_pool(name="fin", bufs=2))

            # ---- sigma -> s_inv = 1/(relu(2*sigma)+eps) broadcast [128,1]
            s_inv = const_p.tile([128, 1], F32)
            stmp = const_p.tile([128, 1], F32)
            nc.sync.dma_start(stmp[:], _vp(sig.ap(), [(0, 128), (1, 1)], 0))
            nc.vector.tensor_scalar_mul(stmp[:], stmp[:], 2.0)
            nc.vector.tensor_scalar_max(stmp[:], stmp[:], 0.0)
            nc.vector.tensor_scalar_add(stmp[:], stmp[:], 1e-6)
            nc.vector.reciprocal(s_inv[:], stmp[:])

            NYF = YN_K * YN_W
            for s in [st for _ in range(repeat) for st in range(NSTEP)]:
                # ---- rgbyn [128, 3*15*152]: pure-input rgb neighborhoods
                # (bufs=2 and NSTEP=2 -> slots never rewritten: DMAs keep <=1 wait)
                rgbyn = rgb78_p.tile([128, 3 * NYF], DT)
                ynpitch = rgbyn[:].ap[0][0]
                for ch in range(3):
                    for hf in range(2):
                        src = _vp(rgbx.ap(),
                                  [(RGBX_W, 64), (RGBX_W, YN_K), (1, YN_W)],
                                  ch * RGBX_P + 256 * hf + 128 * s)
                        dst = _vp(rgbyn[:], [(ynpitch, 64), (YN_W, YN_K), (1, YN_W)],
                                  64 * hf * ynpitch + ch * NYF)
                        nc.sync.dma_start(dst, src)
                # ---- yn [128, 15*152] = luma(rgbyn), in-place accumulate
                yn = yn_p.tile([128, NYF], DT)
                nc.vector.tensor_scalar_mul(yn[:], rgbyn[:, 0:NYF], 0.299)
                nc.vector.scalar_tensor_tensor(yn[:], rgbyn[:, NYF:2 * NYF],
                                               0.587, yn[:], ALU.mult, ALU.add)
                nc.vector.scalar_tensor_tensor(yn[:], rgbyn[:, 2 * NYF:3 * NYF],
                                               0.114, yn[:], ALU.mult, ALU.add)
                # ---- rgbn [128, 3*11*138]
                rgbn = rgbn_p.tile([128, 3 * RN_K * RN_W], DT)
                rnpitch = rgbn[:].ap[0][0]
                for ch in range(3):
                    for hf in range(2):
                        src = _vp(rgbx.ap(),
                                  [(RGBX_W, 64), (RGBX_W, RN_K), (1, RN_W)],
                                  ch * RGBX_P + 2 * RGBX_W + 7 + 256 * hf + 128 * s)
                        dst = _vp(rgbn[:],
                                  [(rnpitch, 64), (RN_W, RN_K), (1, RN_W)],
                                  64 * hf * rnpitch + ch * RN_K * RN_W)
                        nc.sync.dma_start(dst, src)

                # ---- phase A: all group distance maps -> Dbuf
                dbuf = dbuf_p.tile([128, NMAPS * D_W], DT)
                gbase = 0
                for (ar, ax0, nwin) in GROUPS:
                    nd2 = nwin * D2_W
                    rb = box_p.tile([128, nd2], DT, tag="boxtmp")
                    for br in range(5):
                        # diff[j, x2] = yn[k=br+5, x2+5..] - yn[k=br+5+ar, x2+5+ax_j..]
                        diff = diff_p.tile([128, nd2], DT)
                        in0 = _vp(yn[:], [(yn[:].ap[0][0], 128), (0, nwin), (1, D2_W)],
                                  (br + 5) * YN_W + 5)
                        in1 = _vp(yn[:], [(yn[:].ap[0][0], 128), (1, nwin), (1, D2_W)],
                                  (br + 5 + ar) * YN_W + 5 + ax0)
                        nc.vector.tensor_tensor(diff[:], in0, in1, ALU.subtract)
                        if br == 0:
                            nc.scalar.activation(rb[:], diff[:], ACTF.Square,
                                                 scale=s_inv[:])
                        else:
                            d2t = d2_p.tile([128, nd2], DT)
                            nc.scalar.activation(d2t[:], diff[:], ACTF.Square,
                                                 scale=s_inv[:])
                            nc.vector.tensor_tensor(rb[:], rb[:], d2t[:], ALU.add)
                    # hbox tree over x offsets 0..4 (window stride D2_W -> D_W)
                    def rbw(off):
                        return _vp(rb[:], [(rb[:].ap[0][0], 128), (D2_W, nwin),
                                           (1, D_W)], off)
                    u1 = box_p.tile([128, nwin * D_W], DT, tag="hbtmp")
                    u2 = box_p.tile([128, nwin * D_W], DT, tag="hbtmp")
                    nc.vector.tensor_tensor(u1[:], rbw(0), rbw(4), ALU.add)
                    nc.gpsimd.tensor_tensor(u2[:], rbw(1), rbw(3), ALU.add)
                    nc.vector.tensor_tensor(u1[:], u1[:], rbw(2), ALU.add)
                    dslice = dbuf[:, gbase * D_W:(gbase + nwin) * D_W]
                    nc.vector.tensor_tensor(dslice, u1[:], u2[:], ALU.add)
                    gbase += nwin

                # ---- sqrt + exp (chunked, in-place; all sqrts before all exps
                # to avoid ACT table-set thrash)
                bnds = [0, 33 * D_W, 66 * D_W, 99 * D_W, NMAPS * D_W]
                for lo, hi in zip(bnds[:-1], bnds[1:]):
                    nc.scalar.activation(dbuf[:, lo:hi], dbuf[:, lo:hi], ACTF.Sqrt)
                for lo, hi in zip(bnds[:-1], bnds[1:]):
                    nc.scalar.activation(dbuf[:, lo:hi], dbuf[:, lo:hi],
                                         ACTF.Exp, scale=-1.0)

                # ---- phase C: accumulate num/den
                num = acc_p.tile([128, 3 * CHUNK], F32)
                den = acc_p.tile([128, CHUNK], F32)
                dstep = dbuf[:].ap[0][0]
                # den: one windowed reduce over all 121 maps
                din = _vp(dbuf[:], [(dstep, 128), (1, CHUNK), (D_W, NMAPS)], 5)
                nc.vector.tensor_reduce(den[:], din, mybir.AxisListType.X, ALU.add)
                # num: per channel, products of all maps into Tall, one reduce
                for ch in range(3):
                    tall = t_p.tile([128, NMAPS * CHUNK], DT)
                    tpitch = tall[:].ap[0][0]
                    gbase = 0
                    for (ar, ax0, nwin) in GROUPS:
                        w_in = _vp(dbuf[:], [(dstep, 128), (D_W, nwin), (1, CHUNK)],
                                   gbase * D_W + 5)
                        r_in = _vp(rgbn[:], [(rgbn[:].ap[0][0], 128), (1, nwin),
                                             (1, CHUNK)],
                                   (ch * RN_K + 5 + ar) * RN_W + 5 + ax0)
                        tsl = tall[:, gbase * CHUNK:(gbase + nwin) * CHUNK]
                        nc.vector.tensor_tensor(tsl, w_in, r_in, ALU.mult)
                        gbase += nwin
                    tin = _vp(tall[:], [(tpitch, 128), (1, CHUNK), (CHUNK, NMAPS)], 0)
                    nc.vector.tensor_reduce(num[:, ch * CHUNK:(ch + 1) * CHUNK],
                                            tin, mybir.AxisListType.X, ALU.add)

                # ---- finalize: out = clip(num/den, 0, 1), merged store
                rec = fin_p.tile([128, CHUNK], F32)
                nc.vector.reciprocal(rec[:], den[:])
                prod = fin_p.tile([128, 3 * CHUNK], F32)
                for ch in range(3):
                    psl = prod[:, ch * CHUNK:(ch + 1) * CHUNK]
                    nc.vector.tensor_tensor(psl,
                                            num[:, ch * CHUNK:(ch + 1) * CHUNK],
                                            rec[:], ALU.mult)
                    nc.vector.tensor_scalar(psl, psl, 0.0, 1.0,
                                            ALU.max, ALU.min)
                # one store per x-half: dst dims (row, ch, x) matches src
                pstep = prod[:].ap[0][0]
                for hf in range(2):
                    dst = _vp(outd.ap(), [(W, 64), (64 * W, 3), (1, CHUNK)],
                              256 * hf + 128 * s)
                    src = _vp(prod[:], [(pstep, 64), (CHUNK, 3), (1, CHUNK)],
                              64 * hf * pstep)
                    nc.gpsimd.dma_start(dst, src)
    nc.compile()
    return nc


def host_prepare(rgb, sigma):
    rgb = np.asarray(rgb, dtype=np.float32)[0]
    sigma = np.asarray(sigma, dtype=np.float32).reshape(1, 1)
    in_maps = []
    for c in range(NCORES):
        rows = (np.arange(64 * c - 7, 64 * c + 71)) % H
        cols = (np.arange(-12, W + 12)) % W
        rgbx = np.ascontiguousarray(rgb[:, rows[:, None], cols[None, :]])
        in_maps.append({"rgbx": rgbx.astype(np.float16), "sigma": sigma})
    return in_maps


_CACHE = {}


def kernel(rgb, sigma):
    if "nc" not in _CACHE:
        _CACHE["nc"] = build_bass()
    nc = _CACHE["nc"]
    in_maps = host_prepare(rgb, sigma)
    res = run_bass_kernel_spmd(nc, in_maps, core_ids=list(range(NCORES)))
    out = np.concatenate([res.results[c]["out"] for c in range(NCORES)], axis=1)
    return out[None].astype(np.float32)


# revision 52
# speedup vs baseline: 1.1443x; 1.1443x over previous
"""Self-contained Trainium2 Bass kernel for NonLocalMeansFast.

Full inputs in, full output out; internally shards across 8 NeuronCores.

Strategy (stripe layout, all compute lane-local):
  - core c owns output rows [64c, 64c+64)
  - partitions p in 0..127: row r = 64c + p%64, x-half = p//64
  - 2 chunk-steps over x: xbase(p,s) = 256*(p//64) + 128*s, 128 cols each
  - per-partition DMA-replicated neighborhoods (y: 15 rows x 152, rgb: 11 x 138)
  - all 121 shifts: weights w_a = exp(-sqrt(box5((y - shift_a(y))^2))/h)
    computed as windowed wide-instruction groups (11 x-shifts per instruction)
  - VectorE: diffs/box-trees/products/reduces; ScalarE: square/sqrt/exp
"""

import sys
import os
import numpy as np

for _p in ("/opt/trn_rl_repo", "/root/.axon_site/_ro/trn_rl_repo"):
    if os.path.isdir(_p) and _p not in sys.path:
        sys.path.insert(0, _p)

import concourse.bass as bass
import concourse.bacc as bacc
import concourse.tile as tile
from concourse import mybir
from concourse.bass_utils import run_bass_kernel_spmd

SR, PR = 5, 2
H = W = 512
NCORES = 8
CHUNK = 128
NSTEP = 2
YN_W = CHUNK + 2 * (SR + PR + SR)     # 152
YN_K = 2 * (SR + PR) + 1              # 15
RN_W = CHUNK + 2 * SR                 # 138
RN_K = 2 * SR + 1                     # 11
D_W = CHUNK + 2 * SR                  # 138
D2_W = CHUNK + 2 * (SR + PR)          # 142
RGBX_W = W + 24                       # 536
RGBX_K = 64 + 14                      # 78

# 11 groups of 11 x-shift windows; ar=0 includes the (0,0) center shift,
# whose map is exactly 0 -> w = exp(0) = 1, matching the reference's center.
GROUPS = [(ar, -SR, 2 * SR + 1) for ar in range(-SR, SR + 1)]
NMAPS = sum(g[2] for g in GROUPS)     # 121

F32 = mybir.dt.float32
DT = mybir.dt.float16                 # compute dtype for maps/products
ALU = mybir.AluOpType
ACTF = mybir.ActivationFunctionType


def _vp(ap, dims, offset):
    """Return a copy of `ap` with a custom [step,count] pattern + offset."""
    c = ap.copy()
    c.ap = mybir.VecI64Pair([(int(s), int(n)) for s, n in dims])
    c.offset = int(offset)
    return c


def build_bass(repeat=1):
    nc = bacc.Bacc("TRN2", target_bir_lowering=False, debug=False)

    rgbx = nc.dram_tensor("rgbx", [3, RGBX_K, RGBX_W], DT, kind="ExternalInput")
    sig = nc.dram_tensor("sigma", [1, 1], F32, kind="ExternalInput")
    outd = nc.dram_tensor("out", [3, 64, W], F32, kind="ExternalOutput")

    RGBX_P = RGBX_K * RGBX_W  # flat pitch of one channel in rgbx

    with tile.TileContext(nc) as tc:
        import contextlib
        with contextlib.ExitStack() as ctx:
            const_p = ctx.enter_context(tc.tile_pool(name="const", bufs=1))
            rgb78_p = ctx.enter_context(tc.tile_pool(name="rgb78", bufs=2))
            y78_p = ctx.enter_context(tc.tile_pool(name="y78", bufs=1))
            yn_p = ctx.enter_context(tc.tile_pool(name="yn", bufs=2))
            rgbn_p = ctx.enter_context(tc.tile_pool(name="rgbn", bufs=2))
            diff_p = ctx.enter_context(tc.tile_pool(name="diff", bufs=3))
            d2_p = ctx.enter_context(tc.tile_pool(name="d2", bufs=3))
            box_p = ctx.enter_context(tc.tile_pool(name="box", bufs=3))
            dbuf_p = ctx.enter_context(tc.tile_pool(name="dbuf", bufs=2))
            t_p = ctx.enter_context(tc.tile_pool(name="tt", bufs=1))
            red_p = ctx.enter_context(tc.tile_pool(name="red", bufs=1))
            acc_p = ctx.enter_context(tc.tile_pool(name="acc", bufs=1))
            fin_p = ctx.enter_context(tc.tile_pool(name="fin", bufs=1))

            # ---- sigma -> s_inv = 1/(relu(2*sigma)+eps) broadcast [128,1]
            s_inv = const_p.tile([128, 1], F32)
            stmp = const_p.tile([128, 1], F32)
            nc.sync.dma_start(stmp[:], _vp(sig.ap(), [(0, 128), (1, 1)], 0))
            nc.vector.tensor_scalar_mul(stmp[:], stmp[:], 2.0)
            nc.vector.tensor_scalar_max(stmp[:], stmp[:], 0.0)
            nc.vector.tensor_scalar_add(stmp[:], stmp[:], 1e-6)
            nc.vector.reciprocal(s_inv[:], stmp[:])

            def halving_tree(tile_ap, n, sw, final_out, final_off):
                """In-place pairwise sum of n contiguous sw-wide slots; the
                last add (n=2) writes final_out (f32) reading at +final_off."""
                while n > 2:
                    h = n // 2
                    lo = tile_ap[:, 0:h * sw]
                    hi = tile_ap[:, (n - h) * sw:n * sw]
                    nc.vector.tensor_tensor(lo, lo, hi, ALU.add)
                    n = n - h
                a = _vp(tile_ap, [(tile_ap.ap[0][0], 128), (1, CHUNK)], final_off)
                b = _vp(tile_ap, [(tile_ap.ap[0][0], 128), (1, CHUNK)],
                        sw + final_off)
                nc.vector.tensor_tensor(final_out, a, b, ALU.add)

            def emit_phase_c(s, dbuf, rgbn):
                # ---- phase C: accumulate num/den from dbuf (weights) + rgbn
                num = acc_p.tile([128, 3 * CHUNK], F32)
                den = acc_p.tile([128, CHUNK], F32)
                dstep = dbuf[:].ap[0][0]
                # den: first tree level out-of-place (dbuf still needed), rest
                # in-place in the scratch; all levels fp16 2x contiguous adds
                dtree = red_p.tile([128, 61 * D_W], DT, tag="dtree")
                nc.vector.tensor_tensor(dtree[:, 0:60 * D_W],
                                        dbuf[:, 0:60 * D_W],
                                        dbuf[:, 61 * D_W:121 * D_W], ALU.add)
                nc.vector.tensor_copy(dtree[:, 60 * D_W:61 * D_W],
                                      dbuf[:, 60 * D_W:61 * D_W])
                halving_tree(dtree[:], 61, D_W, den[:], 5)
                for ch in range(3):
                    tall = t_p.tile([128, NMAPS * CHUNK], DT)
                    gbase = 0
                    for gi, (ar, ax0, nwin) in enumerate(GROUPS):
                        w_in = _vp(dbuf[:], [(dstep, 128), (D_W, nwin), (1, CHUNK)],
                                   gbase * D_W + 5)
                        r_in = _vp(rgbn[:], [(rgbn[:].ap[0][0], 128), (1, nwin),
                                             (1, CHUNK)],
                                   (ch * RN_K + 5 + ar) * RN_W + 5 + ax0)
                        tsl = tall[:, gbase * CHUNK:(gbase + nwin) * CHUNK]
                        meng = nc.gpsimd if gi % 4 == 1 else nc.vector
                        meng.tensor_tensor(tsl, w_in, r_in, ALU.mult)
                        gbase += nwin
                    halving_tree(tall[:], NMAPS, CHUNK,
                                 num[:, ch * CHUNK:(ch + 1) * CHUNK], 0)
                # ---- finalize: out = clip(num/den, 0, 1), merged store
                rec = fin_p.tile([128, CHUNK], F32)
                nc.vector.reciprocal(rec[:], den[:])
                prod = fin_p.tile([128, 3 * CHUNK], F32)
                for ch in range(3):
                    psl = prod[:, ch * CHUNK:(ch + 1) * CHUNK]
                    nc.vector.tensor_tensor(psl,
                                            num[:, ch * CHUNK:(ch + 1) * CHUNK],
                                            rec[:], ALU.mult)
                    nc.vector.tensor_scalar(psl, psl, 0.0, 1.0,
                                            ALU.max, ALU.min)
                pstep = prod[:].ap[0][0]
                for hf in range(2):
                    dst = _vp(outd.ap(), [(W, 64), (64 * W, 3), (1, CHUNK)],
                              256 * hf + 128 * s)
                    src = _vp(prod[:], [(pstep, 64), (CHUNK, 3), (1, CHUNK)],
                              64 * hf * pstep)
                    nc.gpsimd.dma_start(dst, src)

            NYF = YN_K * YN_W
            pending = []
            for s in [st for _ in range(repeat) for st in range(NSTEP)]:
                # ---- rgbyn [128, 3*15*152]: pure-input rgb neighborhoods
                # (bufs=2 and NSTEP=2 -> slots never rewritten: DMAs keep <=1 wait)
                rgbyn = rgb78_p.tile([128, 3 * NYF], DT)
                ynpitch = rgbyn[:].ap[0][0]
                for ch in range(3):
                    for hf in range(2):
                        src = _vp(rgbx.ap(),
                                  [(RGBX_W, 64), (RGBX_W, YN_K), (1, YN_W)],
                                  ch * RGBX_P + 256 * hf + 128 * s)
                        dst = _vp(rgbyn[:], [(ynpitch, 64), (YN_W, YN_K), (1, YN_W)],
                                  64 * hf * ynpitch + ch * NYF)
                        nc.sync.dma_start(dst, src)
                # ---- yn [128, 15*152] = luma(rgbyn), in-place accumulate
                yn = yn_p.tile([128, NYF], DT)
                nc.vector.tensor_scalar_mul(yn[:], rgbyn[:, 0:NYF], 0.299)
                nc.vector.scalar_tensor_tensor(yn[:], rgbyn[:, NYF:2 * NYF],
                                               0.587, yn[:], ALU.mult, ALU.add)
                nc.vector.scalar_tensor_tensor(yn[:], rgbyn[:, 2 * NYF:3 * NYF],
                                               0.114, yn[:], ALU.mult, ALU.add)
                # ---- rgbn [128, 3*11*138]
                rgbn = rgbn_p.tile([128, 3 * RN_K * RN_W], DT)
                rnpitch = rgbn[:].ap[0][0]
                for ch in range(3):
                    for hf in range(2):
                        src = _vp(rgbx.ap(),
                                  [(RGBX_W, 64), (RGBX_W, RN_K), (1, RN_W)],
                                  ch * RGBX_P + 2 * RGBX_W + 7 + 256 * hf + 128 * s)
                        dst = _vp(rgbn[:],
                                  [(rnpitch, 64), (RN_W, RN_K), (1, RN_W)],
                                  64 * hf * rnpitch + ch * RN_K * RN_W)
                        nc.sync.dma_start(dst, src)

                # ---- phase A: all group distance maps -> Dbuf
                dbuf = dbuf_p.tile([128, NMAPS * D_W], DT)
                gbase = 0
                for gi, (ar, ax0, nwin) in enumerate(GROUPS):
                    nd2 = nwin * D2_W
                    rb = box_p.tile([128, nd2], DT, tag="boxtmp")
                    for br in range(5):
                        # diff[j, x2] = yn[k=br+5, x2+5..] - yn[k=br+5+ar, x2+5+ax_j..]
                        diff = diff_p.tile([128, nd2], DT)
                        in0 = _vp(yn[:], [(yn[:].ap[0][0], 128), (0, nwin), (1, D2_W)],
                                  (br + 5) * YN_W + 5)
                        in1 = _vp(yn[:], [(yn[:].ap[0][0], 128), (1, nwin), (1, D2_W)],
                                  (br + 5 + ar) * YN_W + 5 + ax0)
                        seng = nc.gpsimd if (br == 2 or (br == 1 and gi % 2 == 0)) \
                            else nc.vector
                        seng.tensor_tensor(diff[:], in0, in1, ALU.subtract)
                        if br == 0:
                            nc.scalar.activation(rb[:], diff[:], ACTF.Square,
                                                 scale=s_inv[:])
                        else:
                            d2t = d2_p.tile([128, nd2], DT)
                            nc.scalar.activation(d2t[:], diff[:], ACTF.Square,
                                                 scale=s_inv[:])
                            nc.vector.tensor_tensor(rb[:], rb[:], d2t[:], ALU.add)
                    # hbox tree over x offsets 0..4 (window stride D2_W -> D_W)
                    def rbw(off):
                        return _vp(rb[:], [(rb[:].ap[0][0], 128), (D2_W, nwin),
                                           (1, D_W)], off)
                    u1 = box_p.tile([128, nwin * D_W], DT, tag="hbtmp")
                    u2 = box_p.tile([128, nwin * D_W], DT, tag="hbtmp")
                    nc.vector.tensor_tensor(u1[:], rbw(0), rbw(4), ALU.add)
                    nc.gpsimd.tensor_tensor(u2[:], rbw(1), rbw(3), ALU.add)
                    nc.vector.tensor_tensor(u1[:], u1[:], rbw(2), ALU.add)
                    dslice = dbuf[:, gbase * D_W:(gbase + nwin) * D_W]
                    nc.vector.tensor_tensor(dslice, u1[:], u2[:], ALU.add)
                    gbase += nwin

                # ---- sqrt + exp (chunked, in-place; all sqrts before all exps
                # to avoid ACT table-set thrash)
                bnds = [0, 33 * D_W, 66 * D_W, 99 * D_W, NMAPS * D_W]
                for lo, hi in zip(bnds[:-1], bnds[1:]):
                    nc.scalar.activation(dbuf[:, lo:hi], dbuf[:, lo:hi], ACTF.Sqrt)
                for lo, hi in zip(bnds[:-1], bnds[1:]):
                    nc.scalar.activation(dbuf[:, lo:hi], dbuf[:, lo:hi],
                                         ACTF.Exp, scale=-1.0)

                # defer phase C: emit it AFTER the next step's phase A so the
                # in-order engine queues overlap the two steps
                pending.append((s, dbuf, rgbn))
                if len(pending) > 1:
                    emit_phase_c(*pending.pop(0))
            while pending:
                emit_phase_c(*pending.pop(0))
    nc.compile()
    return nc


def host_prepare(rgb, sigma):
    rgb = np.asarray(rgb, dtype=np.float32)[0]
    sigma = np.asarray(sigma, dtype=np.float32).reshape(1, 1)
    in_maps = []
    for c in range(NCORES):
        rows = (np.arange(64 * c - 7, 64 * c + 71)) % H
        cols = (np.arange(-12, W + 12)) % W
        rgbx = np.ascontiguousarray(rgb[:, rows[:, None], cols[None, :]])
        in_maps.append({"rgbx": rgbx.astype(np.float16), "sigma": sigma})
    return in_maps


_CACHE = {}


def kernel(rgb, sigma):
    if "nc" not in _CACHE:
        _CACHE["nc"] = build_bass()
    nc = _CACHE["nc"]
    in_maps = host_prepare(rgb, sigma)
    res = run_bass_kernel_spmd(nc, in_maps, core_ids=list(range(NCORES)))
    out = np.concatenate([res.results[c]["out"] for c in range(NCORES)], axis=1)
    return out[None].astype(np.float32)
